# revision 1
# baseline (speedup 1.0000x reference)
"""Trainium2 Bass kernel for nn_MergeMetaCNN (hypernetwork MLP -> grouped conv -> CNN).

Data-parallel over batch: 32 samples -> 8 NeuronCores, 4 samples each.

Per-core pipeline (all math on device):
  1. MLP (fp32 matmuls): hid = relu(W1^T fxT + b1); rawT = W2^T hid + b2,
     scaled per-row by 0.1/27 (filter part) / 0.1 (bias part).
  2. conv1 (grouped 3x3, per-sample dynamic filters) as ONE matmul pass:
     block-diagonal stationary [4*27, 4*8] (bf16), moving operand = im2col
     tile [108, rows, 226] DMA-gathered from zero-padded bf16 X planes
     (each partition = contiguous shifted window of a padded plane).
  3. conv2 (8->64, 3x3) per sample: stationary [72, 64], moving = im2col
     [72, rows, 226] gathered from padded bf16 y planes.
  4. Epilogue relu(x + b) split across ScalarE/VectorE, bf16 staging,
     upcast to fp32 during the store DMA (SWDGE cast).
"""

import numpy as np
from contextlib import ExitStack

import concourse.bass as bass
import concourse.tile as tile
from concourse import bacc, mybir
from concourse.bass_utils import run_bass_kernel_spmd

AP = bass.AP
f32 = mybir.dt.float32
bf16 = mybir.dt.bfloat16
AF = mybir.ActivationFunctionType
ALU = mybir.AluOpType

# Problem constants (hardcoded per contract)
B, CIN, H, W = 32, 3, 224, 224
TMP, K, FLAT, COUT = 8, 3, 128, 64
MLP_OUT = TMP * CIN * K * K + TMP  # 224
META = 0.1
NCORES = 8
SPC = B // NCORES                  # 4 samples per core
PH, PW = H + 2, W + 2              # 226 (zero-pad 1 on each side)
PLANE = PH * PW                    # 51076
PP = PLANE + 4                     # padded plane stride (tail slack for windows)
K27 = CIN * K * K                  # 27
K72 = TMP * K * K                  # 72
RT = 16                            # image rows per row-tile
NRT = H // RT                      # 14 row-tiles
RMM = 2                            # rows per matmul (PSUM bank: 448 fp32 <= 512)
NFREE = RMM * W                    # 448
MM_PER_TILE = RT // RMM            # 4

_CACHE = {}


def build_module(repeat=1, loop_n=None):
    """Build + compile the single-core Bass module (SPMD across 8 cores).

    repeat>1 duplicates the conv pipeline instructions. loop_n wraps the
    pipeline in a hardware For_i loop executing it loop_n times with a
    constant instruction count -- wall-clock slope over loop_n isolates
    device execution time from NEFF load/dispatch overhead."""
    key = ("nc", repeat, loop_n)
    if key in _CACHE:
        return _CACHE[key]
    nc = bacc.Bacc("TRN2", target_bir_lowering=False, debug=False, num_devices=NCORES)

    # ---- DRAM I/O (per-core shapes) ----
    padX = nc.dram_tensor("padX", [SPC * CIN, PP], f32, kind="ExternalInput")
    fxT = nc.dram_tensor("fxT", [FLAT, SPC], f32, kind="ExternalInput")
    W1 = nc.dram_tensor("W1", [FLAT, MLP_OUT], f32, kind="ExternalInput")
    b1 = nc.dram_tensor("b1", [MLP_OUT], f32, kind="ExternalInput")
    W2 = nc.dram_tensor("W2", [MLP_OUT, MLP_OUT], f32, kind="ExternalInput")
    b2 = nc.dram_tensor("b2", [MLP_OUT], f32, kind="ExternalInput")
    cnn_wT = nc.dram_tensor("cnn_wT", [K72, COUT], f32, kind="ExternalInput")
    cnn_b = nc.dram_tensor("cnn_b", [COUT], f32, kind="ExternalInput")
    out = nc.dram_tensor("out", [SPC, COUT, H * W], f32, kind="ExternalOutput")

    padXb = nc.dram_tensor("padXb", [SPC * CIN, PP], bf16)   # bf16 cast of padX
    rawT_d = nc.dram_tensor("rawT_d", [MLP_OUT, SPC], f32)   # MLP out scratch

    with tile.TileContext(nc) as tc, ExitStack() as ctx:
        cpool = ctx.enter_context(tc.tile_pool(name="consts", bufs=1))
        spool = ctx.enter_context(tc.tile_pool(name="stageA", bufs=1))
        mlp_ctx = ExitStack()
        mpsum = mlp_ctx.enter_context(tc.tile_pool(name="mlp_psum", bufs=2, space="PSUM"))

        # ================= Stage A: MLP + weight prep =================
        w1sb = cpool.tile([FLAT, MLP_OUT], f32)
        nc.sync.dma_start(w1sb[:], W1.ap())
        w2a = cpool.tile([128, MLP_OUT], f32)
        nc.sync.dma_start(w2a[:], W2.ap()[0:128, :])
        w2b = cpool.tile([96, MLP_OUT], f32)
        nc.sync.dma_start(w2b[:], W2.ap()[128:224, :])
        fx_sb = cpool.tile([FLAT, SPC], f32)
        nc.sync.dma_start(fx_sb[:], fxT.ap())
        b1a = cpool.tile([128, 1], f32)
        nc.sync.dma_start(b1a[:], b1.ap()[0:128].unsqueeze(1))
        b1b = cpool.tile([96, 1], f32)
        nc.sync.dma_start(b1b[:], b1.ap()[128:224].unsqueeze(1))
        b2a = cpool.tile([128, 1], f32)
        nc.sync.dma_start(b2a[:], b2.ap()[0:128].unsqueeze(1))
        b2b = cpool.tile([96, 1], f32)
        nc.sync.dma_start(b2b[:], b2.ap()[128:224].unsqueeze(1))
        cnnb_sb = cpool.tile([COUT, 1], f32)
        nc.sync.dma_start(cnnb_sb[:], cnn_b.ap().unsqueeze(1))
        lhsT2 = cpool.tile([K72, COUT], bf16)
        nc.gpsimd.dma_start(lhsT2[:], cnn_wT.ap())  # cast f32 -> bf16

        # uniform scale 0.1/27 on all raw rows; bias rows corrected by x27 later
        WSCALE = META / K27
        b2v_a = cpool.tile([128, 1], f32)
        nc.vector.tensor_scalar_mul(b2v_a[:], b2a[:], WSCALE)
        b2v_b = cpool.tile([96, 1], f32)
        nc.vector.tensor_scalar_mul(b2v_b[:], b2b[:], WSCALE)

        # hid^T = relu(W1^T @ fxT + b1)   [224, SPC] in two partition chunks
        ph_a = mpsum.tile([128, SPC], f32, tag="mp")
        nc.tensor.matmul(ph_a[:], lhsT=w1sb[:, 0:128], rhs=fx_sb[:], start=True, stop=True)
        hida = spool.tile([128, SPC], f32)
        nc.scalar.activation(hida[:], ph_a[:], func=AF.Relu, bias=b1a[:])
        ph_b = mpsum.tile([96, SPC], f32, tag="mp")
        nc.tensor.matmul(ph_b[:], lhsT=w1sb[:, 128:224], rhs=fx_sb[:], start=True, stop=True)
        hidb = spool.tile([96, SPC], f32)
        nc.scalar.activation(hidb[:], ph_b[:], func=AF.Relu, bias=b1b[:])

        # raw^T = (W2^T @ hid + b2) * vs   [224, SPC]
        pr_a = mpsum.tile([128, SPC], f32, tag="mp")
        nc.tensor.matmul(pr_a[:], lhsT=w2a[:, 0:128], rhs=hida[:], start=True, stop=False)
        nc.tensor.matmul(pr_a[:], lhsT=w2b[:, 0:128], rhs=hidb[:], start=False, stop=True)
        rawa = spool.tile([128, SPC], f32)
        nc.scalar.activation(rawa[:], pr_a[:], func=AF.Identity, bias=b2v_a[:], scale=WSCALE)
        pr_b = mpsum.tile([96, SPC], f32, tag="mp")
        nc.tensor.matmul(pr_b[:], lhsT=w2a[:, 128:224], rhs=hida[:], start=True, stop=False)
        nc.tensor.matmul(pr_b[:], lhsT=w2b[:, 128:224], rhs=hidb[:], start=False, stop=True)
        rawb = spool.tile([96, SPC], f32)
        nc.scalar.activation(rawb[:], pr_b[:], func=AF.Identity, bias=b2v_b[:], scale=WSCALE)

        nc.sync.dma_start(rawT_d.ap()[0:128, :], rawa[:])
        nc.sync.dma_start(rawT_d.ap()[128:224, :], rawb[:])

        # conv1 stationary: block-diag [4*27, 4*8] bf16, partition order
        # (ky, s, ci, kx): lhsT1[ky*36 + s*9 + ci*3 + kx, s*8 + t] = wt[s][t,ci,ky,kx]
        lhsT1 = cpool.tile([SPC * K27, SPC * TMP], bf16)
        nc.vector.memset(lhsT1[:], 0.0)
        for s in range(SPC):
            for ky in range(K):
                for ci in range(CIN):
                    src = AP(
                        tensor=rawT_d,
                        offset=(ci * K * K + ky * K) * SPC + s,
                        ap=[[SPC, K], [K27 * SPC, TMP]],
                    )
                    p0 = ky * 36 + s * 9 + ci * K
                    nc.gpsimd.dma_start(
                        lhsT1[p0:p0 + K, s * TMP:(s + 1) * TMP], src
                    )
        # conv1 bias vector [32, 1]: bias1[s*8+t] = rawT[216+t, s]
        bias1 = cpool.tile([SPC * TMP, 1], f32)
        for s in range(SPC):
            nc.sync.dma_start(
                bias1[s * TMP:(s + 1) * TMP, :], rawT_d.ap()[216:224, s:s + 1]
            )
        # bias rows need scale 0.1, not 0.1/27 -> multiply by 27
        nc.vector.tensor_scalar_mul(bias1[:], bias1[:], float(K27))

        mlp_ctx.close()  # release MLP PSUM banks for conv pools

        # ================= Stage B prep: padded bf16 planes =================
        # cast whole padded X (incl. zero ring + tail) to bf16
        nc.gpsimd.dma_start(padXb.ap(), padX.ap())
        # padY lives in SBUF: planes (s,t) on 32 partitions, PP bf16 each.
        # conv1 epilogue (ACT) writes the interior directly; zero the ring once.
        ypool_res = ctx.enter_context(tc.tile_pool(name="ypres", bufs=1))
        padY_sb = ypool_res.tile([SPC * TMP, PP], bf16)
        nc.vector.memset(padY_sb[:, 0:PW], 0.0)                    # top row
        nc.vector.memset(padY_sb[:, 225 * PW:PP], 0.0)             # bottom row + tail
        lr = padY_sb[:, PW:225 * PW].rearrange("p (r c) -> p r c", c=PW)
        nc.vector.memset(lr[:, :, 0:1], 0.0)                       # left col
        nc.vector.memset(lr[:, :, 225:226], 0.0)                   # right col

        # ================= Stage B: conv pipeline =================
        ic1 = ctx.enter_context(tc.tile_pool(name="ic1", bufs=2))
        ic2 = ctx.enter_context(tc.tile_pool(name="ic2", bufs=3))
        op_ = ctx.enter_context(tc.tile_pool(name="opool", bufs=3))
        ps1 = ctx.enter_context(tc.tile_pool(name="ps1", bufs=2, space="PSUM"))
        ps2 = ctx.enter_context(tc.tile_pool(name="ps2", bufs=2, space="PSUM"))

        def conv1_iter(r):
            r0 = r * RT
            t1 = ic1.tile([SPC * K27, RT, PW], bf16, name=f"t1_{ep_ctr[0]}_{r}", tag="t1")
            # partition (ky, s, ci, kx) <- padXb plane (s,ci), shifted by ky*PW+kx
            for ky in range(K):
                src = AP(
                    tensor=padXb,
                    offset=(r0 + ky) * PW,
                    ap=[[PP, SPC * CIN], [1, K], [1, RT * PW]],
                )
                nc.sync.dma_start(t1[ky * 36:(ky + 1) * 36], src)
            for jp in range(MM_PER_TILE // 2):
                p1 = ps1.tile([SPC * TMP, 1024], f32,
                              name=f"p1_{ep_ctr[0]}_{r}_{jp}", tag="p1")
                nc.tensor.matmul(
                    p1[:, 0:NFREE], lhsT=lhsT1[:],
                    rhs=t1[:, 4 * jp:4 * jp + 2, 0:W], start=True, stop=True,
                )
                nc.tensor.matmul(
                    p1[:, 512:512 + NFREE], lhsT=lhsT1[:],
                    rhs=t1[:, 4 * jp + 2:4 * jp + 4, 0:W], start=True, stop=True,
                )
                # write y rows (r0+4jp .. +3) straight into padY_sb interior
                dst = AP(
                    tensor=padY_sb.tensor,
                    offset=(1 + r0 + jp * 4) * PW + 1,
                    ap=[[PP, SPC * TMP], [2 * PW, 2], [PW, 2], [1, W]],
                )
                pv = AP(
                    tensor=p1.tensor, offset=0,
                    ap=[[1024, SPC * TMP], [512, 2], [W, 2], [1, W]],
                )
                nc.scalar.activation(dst, pv, func=AF.Identity, bias=bias1[:])

        ep_ctr = [0]
        ACT_EVERY = 5  # ACT gets 2 of every 5 conv2 epilogues

        def conv2_pair(s0, r):
            r0 = r * RT
            for si in range(2):
                s = s0 + si
                osb = op_.tile([COUT, RT * W], f32,
                               name=f"o_{ep_ctr[0]}_{s}_{r}", tag="o")
                t2 = ic2.tile([K72, RT, PW], bf16,
                              name=f"t2_{ep_ctr[0]}_{s}_{r}", tag="t2")
                # partition (dy, t, dx) <- padY_sb plane (s,t), shifted dy*PW+dx
                for dy in range(K):
                    src = AP(
                        tensor=padY_sb.tensor,
                        offset=s * TMP * PP + (r0 + dy) * PW,
                        ap=[[PP, TMP], [1, K], [1, RT * PW]],
                    )
                    nc.sync.dma_start(t2[dy * 24:(dy + 1) * 24], src)
                for jp in range(MM_PER_TILE // 2):
                    # two matmuls into the two banks of one [64, 1024] psum tile
                    p2 = ps2.tile([COUT, 1024], f32,
                                  name=f"p2_{ep_ctr[0]}_{s}_{r}_{jp}", tag="p2")
                    nc.tensor.matmul(
                        p2[:, 0:NFREE], lhsT=lhsT2[:],
                        rhs=t2[:, 4 * jp:4 * jp + 2, 0:W], start=True, stop=True,
                    )
                    nc.tensor.matmul(
                        p2[:, 512:512 + NFREE], lhsT=lhsT2[:],
                        rhs=t2[:, 4 * jp + 2:4 * jp + 4, 0:W], start=True, stop=True,
                    )
                    pv = p2.rearrange("p (a b) -> p a b", a=2)[:, :, 0:NFREE]
                    obase = jp * 2 * NFREE
                    oslice = osb[:, obase:obase + 2 * NFREE].rearrange(
                        "p (a b) -> p a b", a=2)
                    if ep_ctr[0] % ACT_EVERY < 2:  # ACT share of conv2 epilogues
                        nc.scalar.activation(oslice, pv, func=AF.Relu,
                                             bias=cnnb_sb[:])
                    else:
                        nc.vector.tensor_scalar(
                            oslice, pv, cnnb_sb[:], 0.0, op0=ALU.add, op1=ALU.max
                        )
                    ep_ctr[0] += 1
                # fp32 store via HWDGE on the ACT ring (separate from the
                # nc.sync ring carrying im2col loads; no SWDGE in the loop)
                nc.scalar.dma_start(
                    out.ap()[s, :, r0 * W:(r0 + RT) * W], osb[:]
                )

        def pipeline():
            conv1_iter(0)
            conv1_iter(1)
            for r in range(NRT):
                if r + 2 < NRT:
                    conv1_iter(r + 2)
                for s0 in (0, 2):
                    conv2_pair(s0, r)

        if loop_n is not None:
            hints = [mybir.EngineType.PE, mybir.EngineType.Activation,
                     mybir.EngineType.DVE, mybir.EngineType.SP,
                     mybir.EngineType.Pool]
            with tc.For_i(0, loop_n, 1, hint_engines=hints):
                pipeline()
        else:
            for _rep in range(repeat):
                pipeline()

    nc.compile()
    _CACHE[key] = nc
    return nc


def make_in_maps(X, flat_x, W1, b1, W2, b2, cnn_w, cnn_b):
    X = np.asarray(X, np.float32)
    flat_x = np.asarray(flat_x, np.float32)
    W1 = np.asarray(W1, np.float32)
    b1 = np.asarray(b1, np.float32)
    W2 = np.asarray(W2, np.float32)
    b2 = np.asarray(b2, np.float32)
    cnn_w = np.asarray(cnn_w, np.float32)
    cnn_b = np.asarray(cnn_b, np.float32)

    img = np.zeros((B, CIN, PH, PW), np.float32)
    img[:, :, 1:1 + H, 1:1 + W] = X
    Xp = np.zeros((B, CIN, PP), np.float32)
    Xp[:, :, :PLANE] = img.reshape(B, CIN, PLANE)
    fxT_full = np.ascontiguousarray(flat_x.T)                  # [128, 32]
    cnn_wT = np.ascontiguousarray(
        cnn_w.transpose(2, 1, 3, 0).reshape(K72, COUT))        # [72,64] (dy,t,dx,co)

    in_maps = []
    for i in range(NCORES):
        sl = slice(i * SPC, (i + 1) * SPC)
        in_maps.append({
            "padX": np.ascontiguousarray(Xp[sl].reshape(SPC * CIN, PP)),
            "fxT": np.ascontiguousarray(fxT_full[:, sl]),
            "W1": W1, "b1": b1, "W2": W2, "b2": b2,
            "cnn_wT": cnn_wT, "cnn_b": cnn_b,
        })
    return in_maps


def kernel(X, flat_x, W1, b1, W2, b2, cnn_w, cnn_b):
    nc = build_module()
    in_maps = make_in_maps(X, flat_x, W1, b1, W2, b2, cnn_w, cnn_b)
    res = run_bass_kernel_spmd(nc, in_maps, core_ids=list(range(NCORES)))
    outs = [res.results[i]["out"].reshape(SPC, COUT, H, W) for i in range(NCORES)]
    return np.concatenate(outs, axis=0).astype(np.float32)



# revision 32
# speedup vs baseline: 1.6084x; 1.6084x over previous
"""Trainium2 Bass kernel for nn_MergeMetaCNN (hypernetwork MLP -> grouped conv -> CNN).

Data-parallel over batch: 32 samples -> 8 NeuronCores, 4 samples each.

Per-core pipeline (all math on device):
  1. MLP (fp32 matmuls) -> per-sample conv1 filters + biases (as raw^T in DRAM).
  2. conv1 (grouped 3x3) as block-diag matmul: stationary [108, 32] bf16
     (rows = (tap, sample, cin)), moving operand = replicated/shifted bf16 X
     band tiles [108, 18*226] built by ONE sync DMA per 16-row band from the
     padded X planes (each partition = one (tap, plane) shifted window).
     Epilogue (ACT/DVE alternating) writes y rows into padded y planes.
  3. y replicas: one sync DMA per band copies the y planes into Y4
     [128 = (s, dy', t), PLANE] with per-partition row shift dy' in {0..3}.
  4. conv2 (8->64, 3x3) with vertical pixel-pairing: M = 128 = (pix in {0,1},
     cout); contraction K = 32 = (dy', t) per sample (PE row-group s via
     tile_position); the 3 kernel columns accumulate in PSUM as 3 matmuls
     whose rhs APs read Y4 at free-dim offsets dx in {0,1,2} -- NO im2col
     gather DMA at all in the conv2 inner loop.
  5. Epilogue relu(x + b) split across ScalarE/VectorE into bf16 staging;
     stores write bf16 rows (even rows from psum partitions 0:64, odd rows
     from 64:128); host upcasts to fp32.
"""

import numpy as np
from contextlib import ExitStack

import concourse.bass as bass
import concourse.tile as tile
from concourse import bacc, mybir
from concourse.bass_utils import run_bass_kernel_spmd

AP = bass.AP
f32 = mybir.dt.float32
bf16 = mybir.dt.bfloat16
AF = mybir.ActivationFunctionType
ALU = mybir.AluOpType

# Problem constants (hardcoded per contract)
B, CIN, H, W = 32, 3, 224, 224
TMP, K, FLAT, COUT = 8, 3, 128, 64
MLP_OUT = TMP * CIN * K * K + TMP  # 224
META = 0.1
NCORES = 8
SPC = B // NCORES                  # 4 samples per core
PH, PW = H + 2, W + 2              # 226 (zero-pad 1 on each side)
PLANE = PH * PW                    # 51076
PP = PLANE + 4                     # padded plane stride (tail slack)
K27 = CIN * K * K                  # 27
K108 = SPC * K27                   # conv1 contraction (block-diag 4 samples)
RT = 16                            # image rows per band
NRT = H // RT                      # 14 bands
BANDL = (RT + 2) * PW              # conv1 band length incl. 2-row lookahead
YBANDL = (RT + 2) * PW             # conv2 y-replica band length
HWP = H * W

_CACHE = {}


def build_module(repeat=1, loop_n=None):
    key = ("nc", repeat, loop_n)
    if key in _CACHE:
        return _CACHE[key]
    nc = bacc.Bacc("TRN2", target_bir_lowering=False, debug=False, num_devices=NCORES)

    # ---- DRAM I/O (per-core shapes) ----
    padX = nc.dram_tensor("padX", [SPC * CIN + 1, PP], f32, kind="ExternalInput")
    fxT = nc.dram_tensor("fxT", [FLAT, SPC], f32, kind="ExternalInput")
    W1 = nc.dram_tensor("W1", [FLAT, MLP_OUT], f32, kind="ExternalInput")
    b1 = nc.dram_tensor("b1", [MLP_OUT], f32, kind="ExternalInput")
    # W2P: columns 0..215 permuted to q = ci*72 + (dy*3+dx)*8 + t; row 224 = b2
    W2P = nc.dram_tensor("W2P", [MLP_OUT + 1, MLP_OUT], f32, kind="ExternalInput")
    # lhsT1z: zeros [108, 32] staging for the conv1-stationary scatter
    lhsT1z = nc.dram_tensor("lhsT1z", [K108, SPC * TMP], f32, kind="ExternalInput")
    bias_d = nc.dram_tensor("bias_d", [SPC * TMP], f32)   # conv1 bias staging
    cnn_wP = nc.dram_tensor("cnn_wP", [128, 3 * 128], f32, kind="ExternalInput")
    cnn_b128 = nc.dram_tensor("cnn_b128", [128], f32, kind="ExternalInput")
    out = nc.dram_tensor("out", [SPC, COUT, HWP], bf16, kind="ExternalOutput")

    padXb = nc.dram_tensor("padXb", [SPC * CIN + 1, PP], bf16)  # +1 OOB slack

    with tile.TileContext(nc) as tc, ExitStack() as ctx:
        cpool = ctx.enter_context(tc.tile_pool(name="consts", bufs=1))
        spool = ctx.enter_context(tc.tile_pool(name="stageA", bufs=1))
        mlp_ctx = ExitStack()
        mpsum = mlp_ctx.enter_context(tc.tile_pool(name="mlp_psum", bufs=2, space="PSUM"))

        # ================= Stage A: MLP + weight prep =================
        w1sb = cpool.tile([FLAT, MLP_OUT], f32)
        nc.sync.dma_start(w1sb[:], W1.ap())
        w2a = cpool.tile([128, MLP_OUT], f32)
        nc.sync.dma_start(w2a[:], W2P.ap()[0:128, :])
        w2b = cpool.tile([97, MLP_OUT], f32)          # rows 128..224 (incl. b2)
        nc.sync.dma_start(w2b[:], W2P.ap()[128:225, :])
        fx_sb = cpool.tile([FLAT, SPC], f32)
        nc.sync.dma_start(fx_sb[:], fxT.ap())
        b1a = cpool.tile([128, 1], f32)
        nc.sync.dma_start(b1a[:], b1.ap()[0:128].unsqueeze(1))
        b1b = cpool.tile([96, 1], f32)
        nc.sync.dma_start(b1b[:], b1.ap()[128:224].unsqueeze(1))
        cnnb_sb = cpool.tile([128, 1], f32)
        nc.sync.dma_start(cnnb_sb[:], cnn_b128.ap().unsqueeze(1))
        lhsTc = cpool.tile([128, 3 * 128], bf16)
        nc.gpsimd.dma_start(lhsTc[:], cnn_wP.ap())  # cast f32 -> bf16

        WSCALE = META / K27

        # hid^T = relu(W1^T @ fxT + b1)  [224, SPC]; extra ones-row for b2
        ph_a = mpsum.tile([128, SPC], f32, tag="mp")
        nc.tensor.matmul(ph_a[:], lhsT=w1sb[:, 0:128], rhs=fx_sb[:], start=True, stop=True)
        hida = spool.tile([128, SPC], f32)
        nc.scalar.activation(hida[:], ph_a[:], func=AF.Relu, bias=b1a[:])
        ph_b = mpsum.tile([96, SPC], f32, tag="mp")
        nc.tensor.matmul(ph_b[:], lhsT=w1sb[:, 128:224], rhs=fx_sb[:], start=True, stop=True)
        hidb = spool.tile([97, SPC], f32)
        nc.vector.memset(hidb[96:97, :], 1.0)
        nc.scalar.activation(hidb[0:96, :], ph_b[:], func=AF.Relu, bias=b1b[:])

        # raw = (hid^T)^T @ W2P * WSCALE  [SPC, 224]  (sample-major, permuted)
        praw = mpsum.tile([SPC, MLP_OUT], f32, tag="mp")
        nc.tensor.matmul(praw[:], lhsT=hida[:], rhs=w2a[:], start=True, stop=False)
        nc.tensor.matmul(praw[:], lhsT=hidb[:], rhs=w2b[:], start=False, stop=True)
        raw_sb = spool.tile([SPC, MLP_OUT], f32)
        nc.scalar.activation(raw_sb[:], praw[:], func=AF.Identity, scale=WSCALE)

        # conv1 stationary [108, 32] bf16: row (dy*3+dx)*12 + s*3 + ci,
        # col s*8 + t = wt[s][t, ci, dy, dx] * 0.1/27. raw col q = ci*72+dydx*8+t.
        # Scatter through DRAM (host-zeroed) to keep SBUF writes aligned.
        for dydx in range(9):
            nc.sync.dma_start(
                AP(tensor=lhsT1z, offset=dydx * 12 * 32,
                   ap=[[3 * 32 + 8, SPC], [32, CIN], [1, TMP]]),
                AP(tensor=raw_sb.tensor, offset=dydx * TMP,
                   ap=[[MLP_OUT, SPC], [72, CIN], [1, TMP]]),
            )
        lhsT1 = cpool.tile([K108, SPC * TMP], bf16)
        nc.gpsimd.dma_start(lhsT1[:], lhsT1z.ap())  # cast f32 -> bf16

        # conv1 bias [32,1] via DRAM staging: bias_d[s*8+t] = raw[s, 216+t]
        nc.sync.dma_start(
            AP(tensor=bias_d, offset=0, ap=[[TMP, SPC], [1, TMP]]),
            AP(tensor=raw_sb.tensor, offset=216, ap=[[MLP_OUT, SPC], [1, TMP]]),
        )
        bias1 = cpool.tile([SPC * TMP, 1], f32)
        nc.sync.dma_start(bias1[:], bias_d.ap().unsqueeze(1))
        nc.vector.tensor_scalar_mul(bias1[:], bias1[:], float(K27))  # undo /27

        mlp_ctx.close()  # release MLP PSUM banks for conv pools

        # ============ resident buffers ============
        # cast padded X to bf16 in DRAM (xr band gathers read it 9x)
        nc.gpsimd.dma_start(padXb.ap(), padX.ap())

        ybase = cpool.tile([SPC * TMP + 1, PP], bf16)   # (s,t) planes +slack
        nc.vector.memset(ybase[SPC * TMP:SPC * TMP + 1, :], 0.0)
        nc.vector.memset(ybase[0:SPC * TMP, 0:PW], 0.0)             # top row
        nc.vector.memset(ybase[0:SPC * TMP, 225 * PW:PP], 0.0)      # bottom+tail
        lr = ybase[0:SPC * TMP, PW:225 * PW].rearrange("p (r c) -> p r c", c=PW)
        nc.vector.memset(lr[:, :, 0:1], 0.0)                        # left col
        nc.vector.memset(lr[:, :, 225:226], 0.0)                    # right col

        # ============ conv pipeline pools ============
        xrp = ctx.enter_context(tc.tile_pool(name="xrp", bufs=2))
        y4p = ctx.enter_context(tc.tile_pool(name="y4p", bufs=3))
        op_ = ctx.enter_context(tc.tile_pool(name="opool", bufs=4))
        ps1 = ctx.enter_context(tc.tile_pool(name="ps1", bufs=2, space="PSUM"))
        ps2 = ctx.enter_context(tc.tile_pool(name="ps2", bufs=2, space="PSUM"))

        ctr = [0]

        def conv1_iter(r):
            r0 = r * RT
            xr = xrp.tile([K108, BANDL], bf16, name=f"xr_{ctr[0]}_{r}", tag="xr")
            # partition (dy,dx,s,ci) <- padXb plane (s,ci) shifted dy*PW+dx
            for dy in range(K):
                nc.sync.dma_start(
                    xr[dy * 36:(dy + 1) * 36],
                    AP(tensor=padXb, offset=(r0 + dy) * PW,
                       ap=[[1, K], [PP, SPC * CIN], [1, BANDL]]),
                )
            for j in range(4):
                p1 = ps1.tile([SPC * TMP, 1024], f32,
                              name=f"p1_{ctr[0]}_{r}_{j}", tag="p1")
                for u in range(2):
                    nc.tensor.matmul(
                        p1[:, u * 512:u * 512 + 448], lhsT=lhsT1[:],
                        rhs=AP(tensor=xr.tensor, offset=(4 * j + 2 * u) * PW,
                               ap=[[BANDL, K108], [PW, 2], [1, W]]),
                        start=True, stop=True,
                    )
                # write y rows r0+4j .. +3 into ybase interior
                dst = AP(
                    tensor=ybase.tensor,
                    offset=(1 + r0 + 4 * j) * PW + 1,
                    ap=[[PP, SPC * TMP], [2 * PW, 2], [PW, 2], [1, W]],
                )
                pv = AP(
                    tensor=p1.tensor, offset=0,
                    ap=[[1024, SPC * TMP], [512, 2], [W, 2], [1, W]],
                )
                if j % 2 == 0:
                    nc.scalar.activation(dst, pv, func=AF.Identity, bias=bias1[:])
                else:
                    nc.vector.tensor_scalar_add(dst, pv, bias1[:])

        def conv2_group(s, g, yb):
            # two psum tiles = rows 16g..16g+15 of sample s
            pk = []
            for k in range(2):
                pk.append(ps2.tile([128, 1024], f32,
                                   name=f"p2_{ctr[0]}_{s}_{g}_{k}", tag="p2"))
            for dx in range(3):
                lw = lhsTc[s * 32:(s + 1) * 32, dx * 128:(dx + 1) * 128]
                for k in range(2):
                    for bk in range(2):
                        Rl = 8 * k + 4 * bk
                        nc.tensor.matmul(
                            pk[k][:, bk * 512:bk * 512 + 448], lhsT=lw,
                            rhs=AP(tensor=yb.tensor,
                                   offset=(s * 32) * YBANDL + Rl * PW + dx,
                                   ap=[[YBANDL, 32], [2 * PW, 2], [1, W]]),
                            start=(dx == 0), stop=(dx == 2),
                            tile_position=(s * 32, 0),
                        )
            for k in range(2):
                R = 16 * g + 8 * k
                osb = op_.tile([128, 896], bf16, name=f"o_{ctr[0]}_{s}_{g}_{k}", tag="o")
                pv = AP(tensor=pk[k].tensor, offset=0,
                        ap=[[1024, 128], [512, 2], [W, 2], [1, W]])
                ov = AP(tensor=osb.tensor, offset=0,
                        ap=[[896, 128], [448, 2], [W, 2], [1, W]])
                if (s + k) % 2 == 0:
                    nc.scalar.activation(ov, pv, func=AF.Relu, bias=cnnb_sb[:])
                else:
                    nc.vector.tensor_scalar(ov, pv, cnnb_sb[:], 0.0,
                                            op0=ALU.add, op1=ALU.max)
                # even rows (pix=0) from partitions 0:64, odd from 64:128
                for pix in range(2):
                    nc.scalar.dma_start(
                        AP(tensor=out, offset=s * COUT * HWP + (R + pix) * W,
                           ap=[[HWP, COUT], [2 * W, 4], [1, W]]),
                        AP(tensor=osb.tensor, offset=pix * 64 * 896,
                           ap=[[896, COUT], [W, 4], [1, W]]),
                    )

        def pipeline():
            for r in range(NRT):
                conv1_iter(r)
            for g in range(NRT):
                # y replica band: yb[(s,dy',t), e] = ybase[(s,t), 16g*PW+e+dy'*PW]
                # yb partition (s, dy', t) = s*32 + dy'*8 + t; per-(dy',s) copy
                # writes a contiguous 8-partition block
                yb = y4p.tile([128, YBANDL], bf16, name=f"yb_{ctr[0]}_{g}", tag="yb")
                for dyp in range(4):
                    # clamp to stay inside ybase's plane columns; conv2 reads
                    # at most 14*PW+225=3389 elements into any band
                    ln = min(YBANDL, PP - (16 * g + dyp) * PW)
                    for s in range(SPC):
                        nc.sync.dma_start(
                            AP(tensor=yb.tensor,
                               offset=(s * 32 + dyp * TMP) * YBANDL,
                               ap=[[YBANDL, TMP], [1, ln]]),
                            AP(tensor=ybase.tensor,
                               offset=s * TMP * PP + (16 * g + dyp) * PW,
                               ap=[[PP, TMP], [1, ln]]),
                        )
                for s in range(SPC):
                    conv2_group(s, g, yb)
            ctr[0] += 1

        if loop_n is not None:
            hints = [mybir.EngineType.PE, mybir.EngineType.Activation,
                     mybir.EngineType.DVE, mybir.EngineType.SP,
                     mybir.EngineType.Pool]
            with tc.For_i(0, loop_n, 1, hint_engines=hints):
                pipeline()
        else:
            for _rep in range(repeat):
                pipeline()

    nc.compile()
    _CACHE[key] = nc
    return nc


def make_in_maps(X, flat_x, W1, b1, W2, b2, cnn_w, cnn_b):
    X = np.asarray(X, np.float32)
    flat_x = np.asarray(flat_x, np.float32)
    W1 = np.asarray(W1, np.float32)
    b1 = np.asarray(b1, np.float32)
    W2 = np.asarray(W2, np.float32)
    b2 = np.asarray(b2, np.float32)
    cnn_w = np.asarray(cnn_w, np.float32)
    cnn_b = np.asarray(cnn_b, np.float32)

    img = np.zeros((B, CIN, PH, PW), np.float32)
    img[:, :, 1:1 + H, 1:1 + W] = X
    Xp = np.zeros((B, CIN, PP), np.float32)
    Xp[:, :, :PLANE] = img.reshape(B, CIN, PLANE)
    fxT_full = np.ascontiguousarray(flat_x.T)                  # [128, 32]

    # W2 columns permuted: new col q = ci*72 + (dy*3+dx)*8 + t <- old
    # t*27 + ci*9 + dy*3 + dx (bias cols 216..223 unpermuted); b2 appended.
    perm = np.arange(MLP_OUT)
    for t in range(TMP):
        for ci in range(CIN):
            for dydx in range(9):
                perm[ci * 72 + dydx * 8 + t] = t * 27 + ci * 9 + dydx
    W2P = np.zeros((MLP_OUT + 1, MLP_OUT), np.float32)
    W2P[:MLP_OUT, :] = W2[:, perm]
    W2P[MLP_OUT, :] = b2[perm]
    lhsT1z = np.zeros((K108, SPC * TMP), np.float32)

    # conv2 stationary with vertical pixel-pairing:
    # base[dy'*8+t, dx, pix*64+co] = cnn_w[co, t, dy'-pix, dx] (valid dy'-pix)
    base = np.zeros((32, 3, 128), np.float32)
    for dyp in range(4):
        for pix in range(2):
            dy = dyp - pix
            if 0 <= dy <= 2:
                for t in range(TMP):
                    base[dyp * 8 + t, :, pix * 64:(pix + 1) * 64] = \
                        cnn_w[:, t, dy, :].T
    cnn_wP = np.tile(base.reshape(32, 3 * 128), (4, 1))        # [128, 384]
    cnn_b128 = np.tile(cnn_b, 2)                               # [128]

    in_maps = []
    for i in range(NCORES):
        sl = slice(i * SPC, (i + 1) * SPC)
        padx_i = np.zeros((SPC * CIN + 1, PP), np.float32)
        padx_i[:SPC * CIN] = Xp[sl].reshape(SPC * CIN, PP)
        in_maps.append({
            "padX": padx_i,
            "fxT": np.ascontiguousarray(fxT_full[:, sl]),
            "W1": W1, "b1": b1, "W2P": W2P, "lhsT1z": lhsT1z,
            "cnn_wP": cnn_wP, "cnn_b128": cnn_b128,
        })
    return in_maps


def kernel(X, flat_x, W1, b1, W2, b2, cnn_w, cnn_b):
    nc = build_module()
    in_maps = make_in_maps(X, flat_x, W1, b1, W2, b2, cnn_w, cnn_b)
    res = run_bass_kernel_spmd(nc, in_maps, core_ids=list(range(NCORES)))
    outs = [np.asarray(res.results[i]["out"]).astype(np.float32).reshape(
        SPC, COUT, H, W) for i in range(NCORES)]
    return np.concatenate(outs, axis=0)
